# revision 1
# baseline (speedup 1.0000x reference)
"""DKT model (2-layer LSTM + FC + sigmoid) as a Bass/Tile kernel for 8
Trainium2 NeuronCores, data-parallel over the batch dim (64 -> 8 per core).

Structure per core (everything "transposed": hidden/gate index on SBUF
partitions, batch on the free dim, so ACT/DVE use all 128 lanes):

  - One-hot @ W_ih0 is an embedding lookup: gather columns of
    (W_ih0 + b_ih0 + b_hh0) from a DRAM table via indirect DMA (row-major
    per token), DMA-transpose to gate-major layout, spill to DRAM, stream
    back during the recurrence.
  - LSTM recurrence: gates^T = X^T[t] + W_hh @ h^T as 64 [128x128]x[128x8]
    matmuls per step (weights stationary), ACT sigmoid/tanh + DVE cell
    update in [128, 32] tiles, h written straight into an SBUF history
    (H1T/H2T) that doubles as next step's matmul rhs.
  - Layer-1 input matmul X1 = h1 @ W_ih1^T + b1 computed batched per
    64-step chunk (PE-efficient, N=512).
  - FC + sigmoid batched at the end.

Weight dtype knob: bf16 (default) or fp8 e4m3 with x64 scaling for the
recurrent weights (halves PE weight-load time; gates are descaled inside
the ACT ops' `scale`).
"""
import os
import numpy as np
import ml_dtypes

import concourse.bass as bass
import concourse.mybir as mybir
import concourse.tile as tile
import concourse.tile as tile_mod
from concourse.bass import ds
from concourse.vector_clock import ScopedClock

BF16 = mybir.dt.bfloat16
FP8 = mybir.dt.float8e4
F32 = mybir.dt.float32
I32 = mybir.dt.int32
AF = mybir.ActivationFunctionType
ALU = mybir.AluOpType

np_bf16 = ml_dtypes.bfloat16
np_fp8 = ml_dtypes.float8_e4m3

P = 128
H = 512          # hidden
G = 2048         # 4*H gates
NSK = 1024       # n_skills
S = int(os.environ.get('DKT_S', '512'))  # seq len
B = 64           # full batch
NCORES = 8
BC = B // NCORES          # batch per core = 8
T = S * BC                # tokens per core = 4096
TC = P                    # tokens per gather chunk (= 16 steps)
NCHUNK = T // TC          # 32
KCH = 4                   # K chunks of 128 over hidden 512
MCH = 16                  # M chunks of 128 over gates 2048

# ---------------------------------------------------------------------------
# Toolchain workarounds (this walrus build rejects >1 semaphore wait per
# instruction, and Tile's exit drain / loop machinery emit several).

_MAX_WAITS = 1
_MAX_INC = 16
_wctr = [0]


def _split_waits(nc):
    for f in nc.m.functions:
        for bb in f.blocks:
            insts = bb.instructions
            i = 0
            while i < len(insts):
                inst = insts[i]
                si = inst.sync_info
                if si is not None and len(si.on_wait) > _MAX_WAITS:
                    waits = list(si.on_wait)
                    keep = waits[-_MAX_WAITS:]
                    extra = waits[:-_MAX_WAITS]
                    inst.sync_info = mybir.SyncInfo(
                        on_wait=keep, on_update=list(si.on_update)
                    )
                    si = inst.sync_info
                    pos = i
                    for j in range(0, len(extra), _MAX_WAITS):
                        _wctr[0] += 1
                        nop = mybir.InstNoOp(
                            name=f"wsplit_{_wctr[0]}", engine=inst.engine
                        )
                        nop.sync_info = mybir.SyncInfo(
                            on_wait=extra[j : j + _MAX_WAITS], on_update=[]
                        )
                        insts.insert(pos, nop)
                        pos += 1
                        i += 1
                # Non-NoOp instructions reject large sem-update immediates in
                # this walrus; move those updates intact onto an adjacent NoOp
                # (Tile's own stage-reset NoOps carry 1000+ values fine).
                # Never alter values: sem-wr-imm is an absolute write.
                if (
                    si is not None
                    and not isinstance(inst, mybir.InstNoOp)
                    and any(
                        (u.update_value or 0) > _MAX_INC for u in si.on_update
                    )
                ):
                    keep_ups, extras = [], []
                    for u in si.on_update:
                        if (u.update_value or 0) > _MAX_INC:
                            extras.append(u)
                        else:
                            keep_ups.append(u)
                    inst.sync_info = mybir.SyncInfo(
                        on_wait=list(si.on_wait), on_update=keep_ups
                    )
                    is_branch = isinstance(
                        inst,
                        (
                            mybir.InstCompareAndBranch,
                            mybir.InstUnconditionalBranch,
                        ),
                    )
                    pos = i if is_branch else i + 1
                    for u in extras:
                        _wctr[0] += 1
                        nop = mybir.InstNoOp(
                            name=f"isplit_{_wctr[0]}", engine=inst.engine
                        )
                        nop.sync_info = mybir.SyncInfo(on_wait=[], on_update=[u])
                        insts.insert(pos, nop)
                        pos += 1
                        if is_branch:
                            i += 1
                i += 1


def _patched_drain_and_barrier(self, tick_clock, wait_clock):
    nc = self.nc
    drain_inst = nc.sync.drain()
    wait_clock.add_sem_waits(
        drain_inst.ins, ScopedClock({None: tick_clock.global_clock})
    )
    si = drain_inst.ins.sync_info
    if si is not None and len(si.on_wait) > 1:
        waits = list(si.on_wait)
        drain_inst.ins.sync_info = mybir.SyncInfo(
            on_wait=waits[:1], on_update=list(si.on_update)
        )
        for k in range(1, len(waits)):
            nop = nc.sync.nop()
            nop.ins.sync_info = mybir.SyncInfo(on_wait=[waits[k]], on_update=[])
    nc.all_engine_barrier()
    assert self.sems is not None
    popped = nc._tile_sem_poison_stack.pop()
    assert popped is self._sem_poison
    nc.clear_and_free_semaphores(list(self.sems.allocated().values()))
    nc.all_engine_barrier()


tile_mod.TileContext._drain_and_barrier = _patched_drain_and_barrier

# ---------------------------------------------------------------------------


def _build(use_fp8: bool, split: bool = True):
    wdt = FP8 if use_fp8 else BF16
    scale = 64.0 if use_fp8 else 1.0
    inv_scale = 1.0 / scale

    nc = bass.Bass()
    table_d = nc.dram_tensor("table", [G, G], BF16, kind="ExternalInput")
    wh0_d = nc.dram_tensor("wh0", [P, KCH, G], wdt, kind="ExternalInput")
    wh1_d = nc.dram_tensor("wh1", [P, KCH, G], wdt, kind="ExternalInput")
    wi1_d = nc.dram_tensor("wi1", [P, KCH, G], BF16, kind="ExternalInput")
    fcw_d = nc.dram_tensor("fcw", [P, KCH, NSK], BF16, kind="ExternalInput")
    b1_d = nc.dram_tensor("b1", [P, MCH], F32, kind="ExternalInput")
    fcb_d = nc.dram_tensor("fcb", [P, NSK // P], F32, kind="ExternalInput")
    sk_d = nc.dram_tensor("skills", [BC, S], I32, kind="ExternalInput")
    co_d = nc.dram_tensor("corrects", [BC, S], I32, kind="ExternalInput")
    out_d = nc.dram_tensor("out", [BC, S, NSK], F32, kind="ExternalOutput")

    with tile.TileContext(nc) as tc:
        with tc.tile_pool(name="wpool", bufs=1) as wpool, \
             tc.tile_pool(name="hist", bufs=1) as hist, \
             tc.tile_pool(name="work", bufs=1) as work, \
             tc.tile_pool(name="dram", bufs=1, space="DRAM") as dpool, \
             tc.tile_pool(name="x0pool", bufs=1) as x0pool, \
             tc.tile_pool(name="psrec", bufs=2, space="PSUM") as psrec, \
             tc.tile_pool(name="psbig", bufs=2, space="PSUM") as psbig:

            # ---- resident weights ----
            wh0_sb = wpool.tile([P, KCH, G], wdt)
            nc.sync.dma_start(wh0_sb[:], wh0_d[:])
            wh1_sb = wpool.tile([P, KCH, G], wdt)
            nc.sync.dma_start(wh1_sb[:], wh1_d[:])
            wi1_sb = wpool.tile([P, KCH, G], BF16)
            nc.sync.dma_start(wi1_sb[:], wi1_d[:])
            fcw_sb = wpool.tile([P, KCH, NSK], BF16)
            nc.sync.dma_start(fcw_sb[:], fcw_d[:])
            b1_sb = wpool.tile([P, MCH], F32)
            nc.sync.dma_start(b1_sb[:], b1_d[:])
            fcb_sb = wpool.tile([P, NSK // P], F32)
            nc.sync.dma_start(fcb_sb[:], fcb_d[:])

            # ---- token-major idx = skills + 1024*(1 - corrects) ----
            sk_sb = work.tile([P, NCHUNK], I32)
            nc.sync.dma_start(
                sk_sb[:], sk_d[:].rearrange("b (c t) -> t b c", t=TC // BC))
            co_sb = work.tile([P, NCHUNK], I32)
            nc.sync.dma_start(
                co_sb[:], co_d[:].rearrange("b (c t) -> t b c", t=TC // BC))
            idx_sb = work.tile([P, NCHUNK], I32)
            nc.vector.tensor_scalar(
                idx_sb[:], co_sb[:], -1024, 1024, ALU.mult, ALU.add
            )
            nc.vector.tensor_tensor(
                out=idx_sb[:], in0=idx_sb[:], in1=sk_sb[:], op=ALU.add
            )

            # ---- gather + transpose + spill X0T to DRAM (static) ----
            x0t_dram = dpool.tile([NCHUNK + 2, P, MCH, TC], BF16)
            rows = [work.tile([P, G], BF16, name=f"rows{u}") for u in range(2)]
            x0st = [work.tile([P, MCH, TC], BF16, name=f"x0st{u}") for u in range(2)]
            for ch in range(NCHUNK):
                u = ch % 2
                nc.gpsimd.indirect_dma_start(
                    out=rows[u][:], out_offset=None, in_=table_d[:],
                    in_offset=bass.IndirectOffsetOnAxis(
                        ap=idx_sb[:, ch : ch + 1], axis=0
                    ),
                )
                for m in range(MCH):
                    nc.sync.dma_start_transpose(
                        x0st[u][:, m, :], rows[u][:, m * P : (m + 1) * P]
                    )
                nc.sync.dma_start(x0t_dram[ch], x0st[u][:])
            # pad chunks (read by the tail prefetch, never consumed)
            zpad = work.tile([P, MCH, TC], BF16)
            nc.vector.memset(zpad[:], 0.0)
            nc.sync.dma_start(x0t_dram[NCHUNK], zpad[:])
            nc.sync.dma_start(x0t_dram[NCHUNK + 1], zpad[:])

            # ---- histories (token-major, SBUF-resident) ----
            h1t = hist.tile([P, KCH, T], BF16)
            h2t = hist.tile([P, KCH, T], BF16)

            c_ab = [work.tile([P, KCH * BC], F32, name=f"c{u}") for u in range(2)]

            def lstm_step(w_sb, x_ap, h_loc, st, cprev, cnew, sname):
                """One step: gates = x + W_hh@h_prev; update c, h.
                x_ap: [P, MCH, BC] (pre-scaled by `scale`, bias folded);
                h_loc: local history [P, KCH, nsteps*BC + BC] with the
                previous body's last h in slots [0:BC]; step `st` reads
                slots [st*BC : st*BC+BC], writes the next BC."""
                psum = psrec.tile([P, MCH * BC], F32, name=f"ps_{sname}", tag="recps")
                for m in range(MCH):
                    for k in range(KCH):
                        nc.tensor.matmul(
                            psum[:, m * BC : (m + 1) * BC],
                            lhsT=w_sb[:, k, m * P : (m + 1) * P],
                            rhs=h_loc[:, k, st * BC : (st + 1) * BC],
                            start=(k == 0),
                            stop=(k == KCH - 1),
                        )
                gp = work.tile([P, MCH * BC], F32, name=f"gp_{sname}", tag="gp", bufs=2)
                nc.vector.tensor_tensor(
                    out=gp[:].rearrange("p (m b) -> p m b", b=BC),
                    in0=psum[:].rearrange("p (m b) -> p m b", b=BC),
                    in1=x_ap,
                    op=ALU.add,
                )
                ac = work.tile([P, MCH * BC], F32, name=f"ac_{sname}", tag="ac", bufs=2)
                nc.scalar.activation(ac[:, 0:64], gp[:, 0:64], AF.Sigmoid,
                                     scale=inv_scale)
                nc.scalar.activation(ac[:, 64:96], gp[:, 64:96], AF.Tanh,
                                     scale=inv_scale)
                nc.scalar.activation(ac[:, 96:128], gp[:, 96:128], AF.Sigmoid,
                                     scale=inv_scale)
                t1 = work.tile([P, KCH * BC], F32, name=f"t1_{sname}", tag="t1", bufs=2)
                nc.vector.tensor_tensor(out=t1[:], in0=ac[:, 32:64], in1=cprev[:],
                                        op=ALU.mult)
                t2 = work.tile([P, KCH * BC], F32, name=f"t2_{sname}", tag="t2", bufs=2)
                nc.vector.tensor_tensor(out=t2[:], in0=ac[:, 0:32], in1=ac[:, 64:96],
                                        op=ALU.mult)
                nc.vector.tensor_tensor(out=cnew[:], in0=t1[:], in1=t2[:], op=ALU.add)
                th = work.tile([P, KCH * BC], F32, name=f"th_{sname}", tag="th", bufs=2)
                nc.scalar.activation(th[:], cnew[:], AF.Tanh)
                nc.vector.tensor_tensor(
                    out=h_loc[:, :, (st + 1) * BC : (st + 2) * BC],
                    in0=ac[:, 96:128].rearrange("p (k b) -> p k b", b=BC),
                    in1=th[:].rearrange("p (k b) -> p k b", b=BC),
                    op=ALU.mult,
                )

            # ---- layer 0 recurrence: 16 iterations x 2 chunks x 16 steps ----
            nc.vector.memset(c_ab[0][:], 0.0)
            NST0 = 32  # steps per body
            h0_loc = work.tile([P, KCH, (NST0 + 1) * BC], BF16)
            nc.vector.memset(h0_loc[:, :, 0:BC], 0.0)
            x0t = [x0pool.tile([P, MCH, TC], BF16, name=f"x0t{u}") for u in range(2)]
            nc.sync.dma_start(x0t[0][:], x0t_dram[0])
            with tc.For_i(0, NCHUNK // 2, 1, staggered_reset=True) as i:
                nc.sync.dma_start(x0t[1][:], x0t_dram[ds(2 * i + 1, 1)]
                                  .rearrange("a p m t -> (a p) m t"))
                for half in range(2):
                    for u in range(16):
                        st = half * 16 + u  # step within the 2-chunk body
                        lstm_step(
                            wh0_sb,
                            x0t[half][:, :, u * BC : (u + 1) * BC],
                            h0_loc, st,
                            c_ab[st % 2], c_ab[(st + 1) % 2],
                            f"l0_{half}_{u}",
                        )
                nc.sync.dma_start(x0t[0][:], x0t_dram[ds(2 * i + 2, 1)]
                                  .rearrange("a p m t -> (a p) m t"))
                # flush this body's h into the history + carry last h to slot 0
                nc.vector.tensor_copy(
                    h1t[:, :, ds(i * (NST0 * BC), NST0 * BC)],
                    h0_loc[:, :, BC : (NST0 + 1) * BC],
                )
                nc.vector.tensor_copy(
                    h0_loc[:, :, 0:BC], h0_loc[:, :, NST0 * BC : (NST0 + 1) * BC]
                )

            # ---- layer 1: per 64-step chunk, batched X1 then recurrence ----
            nc.vector.memset(c_ab[0][:], 0.0)
            TJ = 512  # tokens per L1 chunk
            NST1 = TJ // BC  # 64 steps per body
            h1_loc = work.tile([P, KCH, (NST1 + 1) * BC], BF16)
            nc.vector.memset(h1_loc[:, :, 0:BC], 0.0)
            x1t = x0pool.tile([P, MCH, TJ], BF16)
            hx = work.tile([P, KCH, TJ], BF16)
            with tc.For_i(0, T // TJ, 1, staggered_reset=True) as j:
                jb = nc.snap(j * TJ)
                nc.vector.tensor_copy(hx[:], h1t[:, :, ds(jb, TJ)])
                for m in range(MCH):
                    psx = psbig.tile([P, TJ], F32, name=f"psx{m}", tag="psx")
                    for k in range(KCH):
                        nc.tensor.matmul(
                            psx[:],
                            lhsT=wi1_sb[:, k, m * P : (m + 1) * P],
                            rhs=hx[:, k, :],
                            start=(k == 0),
                            stop=(k == KCH - 1),
                        )
                    nc.scalar.activation(x1t[:, m, :], psx[:], AF.Identity,
                                         bias=b1_sb[:, m : m + 1], scale=scale)
                for u in range(NST1):
                    lstm_step(
                        wh1_sb,
                        x1t[:, :, u * BC : (u + 1) * BC],
                        h1_loc, u,
                        c_ab[u % 2], c_ab[(u + 1) % 2],
                        f"l1_{u}",
                    )
                nc.vector.tensor_copy(
                    h2t[:, :, ds(jb, TJ)],
                    h1_loc[:, :, BC : (NST1 + 1) * BC],
                )
                nc.vector.tensor_copy(
                    h1_loc[:, :, 0:BC], h1_loc[:, :, NST1 * BC : (NST1 + 1) * BC]
                )

            # ---- FC + sigmoid + store (static) ----
            NT = T // 512   # token chunks of 512
            NM = NSK // P  # 8 logit chunks
            for n in range(NT):
                for m in range(NM):
                    psf = psbig.tile([P, 512], F32, name=f"psf{n}_{m}", tag="psf")
                    for k in range(KCH):
                        nc.tensor.matmul(
                            psf[:],
                            lhsT=fcw_sb[:, k, m * P : (m + 1) * P],
                            rhs=h2t[:, k, n * 512 : (n + 1) * 512],
                            start=(k == 0),
                            stop=(k == KCH - 1),
                        )
                    ot = work.tile([P, BC, 64], F32, name=f"ot{n}_{m}", tag="ot",
                                   bufs=3)
                    nc.scalar.activation(
                        ot[:].rearrange("g b s -> g s b"),
                        psf[:].rearrange("g (s b) -> g s b", b=BC),
                        AF.Sigmoid, bias=fcb_sb[:, m : m + 1])
                    for b in range(BC):
                        nc.sync.dma_start(
                            out_d[b, n * 64 : (n + 1) * 64, m * P : (m + 1) * P]
                            .rearrange("s g -> g s"),
                            ot[:, b, :],
                        )

    if split:
        _split_waits(nc)
    return nc


_cache = {}


def _get_nc(use_fp8):
    if use_fp8 not in _cache:
        _cache[use_fp8] = _build(use_fp8)
    return _cache[use_fp8]


def kernel(skills, corrects, W_ih0, W_hh0, b_ih0, b_hh0,
           W_ih1, W_hh1, b_ih1, b_hh1, fc_W, fc_b):
    use_fp8 = os.environ.get("DKT_WDT", "bf16") == "fp8"
    scale = 64.0 if use_fp8 else 1.0
    np_wdt = np_fp8 if use_fp8 else np_bf16

    skills = np.asarray(skills, np.int32)
    corrects = np.asarray(corrects, np.int32)
    f32 = lambda x: np.asarray(x, np.float32)
    W_ih0, W_hh0, W_ih1, W_hh1, fc_W = map(f32, (W_ih0, W_hh0, W_ih1, W_hh1, fc_W))
    b0 = f32(b_ih0) + f32(b_hh0)
    b1 = f32(b_ih1) + f32(b_hh1)
    fc_b = f32(fc_b)

    table = np.ascontiguousarray(((W_ih0 + b0[:, None]).T * scale).astype(np_bf16))

    def kfmt(w, dt, sc=1.0):  # [G', 512] -> [128, 4, G'] lhsT chunks
        return np.ascontiguousarray(
            (w.T * sc).reshape(KCH, P, w.shape[0]).transpose(1, 0, 2).astype(dt))

    wh0 = kfmt(W_hh0, np_wdt, scale)
    wh1 = kfmt(W_hh1, np_wdt, scale)
    wi1 = kfmt(W_ih1, np_bf16)
    fcw = kfmt(fc_W, np_bf16)
    b1h = np.ascontiguousarray((b1 * scale).reshape(MCH, P).T.astype(np.float32))
    fcb = np.ascontiguousarray(fc_b.reshape(NSK // P, P).T.astype(np.float32))

    nc = _get_nc(use_fp8)

    in_maps = []
    for c in range(NCORES):
        sl = slice(c * BC, (c + 1) * BC)
        in_maps.append({
            "table": table, "wh0": wh0, "wh1": wh1, "wi1": wi1, "fcw": fcw,
            "b1": b1h, "fcb": fcb,
            "skills": np.ascontiguousarray(skills[sl]),
            "corrects": np.ascontiguousarray(corrects[sl]),
        })

    from concourse.bass_utils import run_bass_kernel_spmd
    trace = os.environ.get("DKT_TRACE", "0") == "1"
    if trace:
        import prof_shim
        prof_shim.install()
    res = run_bass_kernel_spmd(nc, in_maps, core_ids=list(range(NCORES)),
                               trace=trace)
    if trace:
        print(f"DKT exec_time_ns: {res.exec_time_ns}")
        kernel.last_result = res
    return np.concatenate([r["out"] for r in res.results], axis=0)

